# revision 21
# baseline (speedup 1.0000x reference)
"""Trainium2 Bass kernel for nn_MultiHeadAttention_44281112822190.

8 NeuronCores, pure data parallelism over the 8192 (b,s) rows: core c takes
rows [c*1024, (c+1)*1024) (batch b = c//2, s-offset (c%2)*1024). No
collectives; the host shards inputs and reassembles the output.

Math notes:
  - The reference applies RoPE to q and k, then contracts q.k at the SAME
    position (per-position head-head attention [B,S,H,H]). RoPE is an
    orthogonal per-position rotation applied identically to q and k, so it
    cancels exactly in the scores: (R q).(R k) = q.k. The kernel skips RoPE
    entirely (freqs inputs are unused).
  - The reference's "h-major flatten" transpose(0,2,1,3).reshape(B,S,-1) is a
    scramble: out[b, h*128 + s//16, (s%16)*128 + d] = att_out[b, s, h, d].
    Each scrambled row draws from 16 consecutive positions of one head, all
    inside one core's shard, so the output projection stays core-local.

Numerics: all matmul operands are fp16 with fp32 PSUM accumulation ->
~7e-4 relative error end-to-end, 1 cycle/row on the PE.

Schedule (v2): the PE streaming floor for the four projections plus
attention is ~464 us/core; this version keeps TensorE dense:
  1. Startup: first wq tile + x^T in 16 fine chunks DMA'd first, weights
     pre-tiled on host so every weight DMA is contiguous (8 KB/partition
     runs). First matmul issues ~5 us in.
  2. Q, K projections: stationary = pre-tiled weight chunks, moving =
     x^T chunks; two N=512 matmuls per LDWEIGHTS; bias added during
     PSUM->SBUF drain. Layout [128 d, 1024 s, 16 h].
  3. V projection in two position-half sweeps (wv streamed twice).
     Attention pairs 0-31 (positions 0-511) are software-pipelined into
     sweep 2's matmul stream in three stages: A = scores matmul +
     exp/mask-sum/reciprocal/normalize, B = att+v transposes (PE), C =
     attO matmul + scatter. Stage lag (A->B 3 slots, B->C 2 slots) covers
     the Scalar/Vector/GpSimd chain latency so TensorE never waits.
  4. Output projection from merged attO halves [128 d, 16 sl, 512] with
     N=512 moving operands, in two sweeps; sweep 1 carries attention
     pairs 32-63 the same way. Drains alternate Vector/GpSimd in
     interleaved phases to keep Vector off the critical path.
Host reassembles the scrambled rows into the final [4, 2048, 2048] output.
"""

import os
import sys

sys.path.insert(0, "/opt/trn_rl_repo")

import numpy as np

import concourse.bacc as bacc
import concourse.mybir as mybir
import concourse.tile as tile
from concourse.bass_utils import run_bass_kernel_spmd

F32 = mybir.dt.float32
F16 = mybir.dt.float16
AF = mybir.ActivationFunctionType
ALU = mybir.AluOpType

B, S, E, H, D = 4, 2048, 2048, 16, 128
NCORES = 8
SCALE = 1.0 / float(np.sqrt(D))

_CACHE = {}
LAST_EXEC_NS = None


def _build():
    nc = bacc.Bacc(trn_type="TRN2", target_bir_lowering=False)

    xt = nc.dram_tensor("xt", [16, 128, 1024], F16, kind="ExternalInput")
    wqt = nc.dram_tensor("wqt", [8, 128, 16, 256], F16, kind="ExternalInput")
    wkt = nc.dram_tensor("wkt", [8, 128, 16, 256], F16, kind="ExternalInput")
    wvt = nc.dram_tensor("wvt", [8, 128, 16, 256], F16, kind="ExternalInput")
    wot = nc.dram_tensor("wot", [8, 128, 16, 256], F16, kind="ExternalInput")
    bqt = nc.dram_tensor("bqt", [128, 16], F32, kind="ExternalInput")
    bkt = nc.dram_tensor("bkt", [128, 16], F32, kind="ExternalInput")
    bvt = nc.dram_tensor("bvt", [128, 16], F32, kind="ExternalInput")
    bot = nc.dram_tensor("bot", [128, 16], F32, kind="ExternalInput")
    mask01 = nc.dram_tensor("mask01", [128, 128], F32, kind="ExternalInput")
    ident = nc.dram_tensor("ident", [128, 128], F16, kind="ExternalInput")
    out = nc.dram_tensor("out", [16, 128, 1024], F32, kind="ExternalOutput")

    with tile.TileContext(nc) as tc:
        with (
            tc.tile_pool(name="const", bufs=1) as cp,
            tc.tile_pool(name="xp", bufs=1) as xp,
            tc.tile_pool(name="qkv", bufs=1) as qkvp,
            tc.tile_pool(name="aop", bufs=1) as aop,
            tc.tile_pool(name="wp", bufs=3) as wp,
            tc.tile_pool(name="gp", bufs=4) as gp,
            tc.tile_pool(name="op", bufs=3) as op,
            tc.tile_pool(name="pp", bufs=4, space="PSUM") as pp,
            tc.tile_pool(name="pa", bufs=3, space="PSUM") as pa,
            tc.tile_pool(name="pb", bufs=1, space="PSUM") as pb,
        ):
            # first weight tile + x first-halves first: the V-A sweep needs
            # only wv[0] + 2 MB of x, so compute starts ~4 us in
            wv_first = wp.tile([128, 16, 256], F16, tag="w")
            nc.sync.dma_start(wv_first[:], wvt[0, :, :, :])
            xa, xb = [], []
            for k in range(16):
                xc = xp.tile([128, 512], F16, tag=f"xa{k}", name=f"xa{k}")
                nc.sync.dma_start(xc[:], xt[k, :, 0:512])
                xa.append(xc)

            mask_sb = cp.tile([128, 128], F32, tag="mask")
            id_sb = cp.tile([128, 128], F16, tag="id")
            nc.sync.dma_start(mask_sb[:], mask01[:, :])
            nc.sync.dma_start(id_sb[:], ident[:, :])
            bias_sb = {}
            for name, t_ in (("bq", bqt), ("bk", bkt), ("bv", bvt), ("bo", bot)):
                b_sb = cp.tile([128, 16], F32, tag=name)
                nc.sync.dma_start(b_sb[:], t_[:, :])
                bias_sb[name] = b_sb

            for k in range(16):
                xc = xp.tile([128, 512], F16, tag=f"xb{k}", name=f"xb{k}")
                nc.sync.dma_start(xc[:], xt[k, :, 512:1024])
                xb.append(xc)

            qb = qkvp.tile([128, 1024, 16], F16, tag="qb")
            kb = qkvp.tile([128, 1024, 16], F16, tag="kb")
            vbA = qkvp.tile([128, 512, 16], F16, tag="vbA")
            vbB = qkvp.tile([128, 512, 16], F16, tag="vbB")
            attO = [
                aop.tile([128, 16, 512], F16, tag=f"attO{i}", name=f"attO{i}")
                for i in range(2)
            ]

            def drain(dst, ps, bias, t, eng):
                # GpSimd cannot read PSUM, so all drains go through Vector.
                nc.vector.tensor_scalar_add(dst, ps, bias_sb[bias][:, t : t + 1])

            def drain4(dst4, ps, bias, t):
                # chunked drain: keeps long Vector ops from delaying the
                # attention softmax chain during interleaved phases
                b_ap = bias_sb[bias][:, t : t + 1]
                for i in range(4):
                    nc.vector.tensor_scalar_add(
                        dst4[i], ps[:, 128 * i : 128 * i + 128], b_ap
                    )

            # --- attention stages -------------------------------------
            astate = {}

            def attn_A(p):
                G = 2 * p
                ga = pa.tile([128, 512], F32, tag="ga")
                for j in range(2):
                    s0 = (G + j) * 8
                    nc.tensor.matmul(
                        ga[:, 128 * j : 128 * j + 128],
                        qb[:, s0 : s0 + 8, :],
                        kb[:, s0 : s0 + 8, :],
                        start=True, stop=True,
                    )
                e2 = gp.tile([128, 256], F32, tag="e2")
                nc.scalar.activation(e2[:], ga[:, 0:256], AF.Exp, scale=SCALE)
                em2 = e2[:].rearrange("p (g c) -> p g c", g=2)
                den2 = gp.tile([128, 2], F32, tag="den2")
                for j, eng in ((0, nc.vector), (1, nc.vector)):
                    eng.scalar_tensor_tensor(
                        em2[:, j, :], e2[:, 128 * j : 128 * j + 128], 1.0,
                        mask_sb[:], ALU.bypass, ALU.mult,
                        accum_out=den2[:, j : j + 1],
                    )
                rec2 = gp.tile([128, 2], F32, tag="rec2")
                nc.vector.reciprocal(rec2[:], den2[:])
                att2 = gp.tile([128, 2, 128], F16, tag="att2")
                nc.gpsimd.tensor_tensor(
                    att2[:], em2,
                    rec2[:].unsqueeze(2).to_broadcast([128, 2, 128]),
                    ALU.mult,
                )
                astate[p] = {"ga": ga, "att2": att2}

            def attn_B(p):
                st = astate[p]
                G = 2 * p
                vb_t, off = (vbA, 0) if p < 32 else (vbB, 512)
                tr = pb.tile([128, 512], F16, tag="tr")
                for j in range(2):
                    s0 = (G + j) * 8 - off
                    nc.tensor.transpose(
                        tr[:, 128 * j : 128 * j + 128], st["att2"][:, j, :],
                        id_sb[:],
                    )
                    nc.tensor.transpose(
                        tr[:, 256 + 128 * j : 384 + 128 * j],
                        vb_t[:, s0 : s0 + 8, :], id_sb[:],
                    )
                trsb = gp.tile([128, 512], F16, tag="trsb")
                nc.scalar.activation(trsb[:], tr[:], AF.Copy)
                st["trsb"] = trsb

            def attn_C(p):
                st = astate.pop(p)
                ga, trsb = st["ga"], st["trsb"]
                for j in range(2):
                    nc.tensor.matmul(
                        ga[:, 256 + 128 * j : 384 + 128 * j],
                        trsb[:, 256 + 128 * j : 384 + 128 * j],
                        trsb[:, 128 * j : 128 * j + 128],
                        start=True, stop=True,
                    )
                hh, u2 = p // 32, p % 32
                dst = attO[hh][:].rearrange(
                    "p (g2 i) (u h) -> p g2 i u h", g2=2, h=16
                )[:, :, :, u2, :]
                src = ga[:, 256:512].rearrange("p (g2 i h) -> p g2 i h", g2=2, i=8)
                nc.scalar.activation(dst, src, AF.Copy)

            # pipelined scheduler: one pump() per 4-matmul slot
            pipe = {"slot": 0, "a": 0, "b": 0, "c": 0,
                    "a_slot": {}, "b_slot": {}, "pairs": []}

            def feed(pairs):
                pipe["pairs"].extend(pairs)

            def pump():
                s = pipe["slot"]
                P = pipe["pairs"]
                if pipe["c"] < pipe["b"]:
                    p = P[pipe["c"]]
                    if pipe["b_slot"][p] <= s - 5:
                        attn_C(p)
                        pipe["c"] += 1
                if pipe["b"] < pipe["a"]:
                    p = P[pipe["b"]]
                    if pipe["a_slot"][p] <= s - 12:
                        attn_B(p)
                        pipe["b_slot"][p] = s
                        pipe["b"] += 1
                if pipe["a"] < len(P) and pipe["a"] - pipe["c"] < 3:
                    p = P[pipe["a"]]
                    attn_A(p)
                    pipe["a_slot"][p] = s
                    pipe["a"] += 1
                pipe["slot"] += 1

            def drain_until(n):
                # hard barrier: all attO writers up to pair n issued before
                # any consumer of their columns is enqueued
                while pipe["c"] < n:
                    pump()

            # --- V sweep A: positions 0-511, dense, runs first (needs only
            # wv[0] + the x first-halves, so compute starts early) ---
            for t2 in range(8):
                if t2 == 0:
                    wtile = wv_first
                else:
                    wtile = wp.tile([128, 16, 256], F16, tag="w")
                    nc.sync.dma_start(wtile[:], wvt[t2, :, :, :])
                for half in range(2):
                    t = 2 * t2 + half
                    ps = pp.tile([128, 512], F32, tag="pp")
                    for k in range(16):
                        nc.tensor.matmul(
                            ps[:], wtile[:, k, half * 128 : half * 128 + 128],
                            xa[k][:],
                            start=(k == 0), stop=(k == 15),
                        )
                    drain(vbA[:, :, t], ps[:], "bv", t, "v")

            # --- Q, K projections -> [128 d, 1024 s, 16 h] fp16 ---
            for wdram, bias, dst in ((wqt, "bq", qb), (wkt, "bk", kb)):
                for t2 in range(8):
                    wtile = wp.tile([128, 16, 256], F16, tag="w")
                    nc.sync.dma_start(wtile[:], wdram[t2, :, :, :])
                    for half in range(2):
                        t = 2 * t2 + half
                        psA = pp.tile([128, 512], F32, tag="pp")
                        psB = pp.tile([128, 512], F32, tag="pp")
                        for k in range(16):
                            w_ap = wtile[:, k, half * 128 : half * 128 + 128]
                            nc.tensor.matmul(
                                psA[:], w_ap, xa[k][:],
                                start=(k == 0), stop=(k == 15),
                            )
                            nc.tensor.matmul(
                                psB[:], w_ap, xb[k][:],
                                start=(k == 0), stop=(k == 15),
                            )
                        drain(dst[:, 0:512, t], psA[:], bias, t, "v")
                        drain(dst[:, 512:1024, t], psB[:], bias, t, "v")

            # --- V sweep B: positions 512-1023, carries attn pairs 0-31 ---
            feed(range(0, 32))
            for t2 in range(8):
                wtile = wp.tile([128, 16, 256], F16, tag="w")
                nc.sync.dma_start(wtile[:], wvt[t2, :, :, :])
                for half in range(2):
                    t = 2 * t2 + half
                    ps = pp.tile([128, 512], F32, tag="pp")
                    for k in range(16):
                        nc.tensor.matmul(
                            ps[:],
                            wtile[:, k, half * 128 : half * 128 + 128],
                            xb[k][:],
                            start=(k == 0), stop=(k == 15),
                        )
                        pump()
                    drain4(
                        [vbB[:, 128 * i : 128 * i + 128, t] for i in range(4)],
                        ps[:], "bv", t,
                    )

            # --- output projection, two sweeps over attO halves ---
            def fin_sweep(hh):
                drain_until(32 * hh + 32)
                if hh == 0:
                    feed(range(32, 64))
                for t2 in range(8):
                    wtile = wp.tile([128, 16, 256], F16, tag="w")
                    nc.sync.dma_start(wtile[:], wot[t2, :, :, :])
                    for half in range(2):
                        t = 2 * t2 + half
                        ps = pp.tile([128, 512], F32, tag="pp")
                        last = hh == 1 and t2 == 7 and half == 1
                        if last:
                            # column-split the final group so its drain and
                            # output DMA overlap the second half's matmuls
                            for colh in range(2):
                                c0 = 256 * colh
                                for sl in range(16):
                                    nc.tensor.matmul(
                                        ps[:, c0 : c0 + 256],
                                        wtile[:, sl, half * 128 : half * 128 + 128],
                                        attO[hh][:, sl, c0 : c0 + 256],
                                        start=(sl == 0), stop=(sl == 15),
                                    )
                                ob = op.tile([128, 256], F32, tag="obh")
                                drain(ob[:], ps[:, c0 : c0 + 256], "bo", t, "v")
                                nc.sync.dma_start(
                                    out[t, :, hh * 512 + c0 : hh * 512 + c0 + 256],
                                    ob[:],
                                )
                            continue
                        for sl in range(16):
                            nc.tensor.matmul(
                                ps[:],
                                wtile[:, sl, half * 128 : half * 128 + 128],
                                attO[hh][:, sl, :],
                                start=(sl == 0), stop=(sl == 15),
                            )
                            pump()
                        ob = op.tile([128, 512], F32, tag="ob")
                        if hh == 0:
                            drain4(
                                [ob[:, 128 * i : 128 * i + 128] for i in range(4)],
                                ps[:], "bo", t,
                            )
                        else:
                            drain(ob[:], ps[:], "bo", t, "v")
                        nc.sync.dma_start(
                            out[t, :, hh * 512 : hh * 512 + 512], ob[:]
                        )

            fin_sweep(0)
            drain_until(64)
            fin_sweep(1)

    nc.compile()
    return nc


def _get_nc():
    if "nc" not in _CACHE:
        _CACHE["nc"] = _build()
    return _CACHE["nc"]


def make_in_maps(inputs):
    x = np.ascontiguousarray(np.asarray(inputs["x"], dtype=np.float32))
    ws = {k: np.asarray(inputs[k], dtype=np.float32) for k in ("wq", "wk", "wv", "wo")}
    bs = {k: np.asarray(inputs[k], dtype=np.float32) for k in ("bq", "bk", "bv", "bo")}

    xf = x.reshape(B * S, E)
    f16 = lambda a: np.ascontiguousarray(a).astype(np.float16)

    def wtileize(w):
        # [t2, p, k, c] tiles: element (p, k, c) = w.T[k*128+p, t2*256+c]
        return np.ascontiguousarray(
            w.T.astype(np.float16).reshape(16, 128, 8, 256).transpose(2, 1, 0, 3)
        )

    btile = lambda b: np.ascontiguousarray(b.reshape(16, 128).T)
    ii = np.arange(128) // 16
    mask01 = (ii[:, None] == ii[None, :]).astype(np.float32)
    common = {
        "wqt": wtileize(ws["wq"]), "wkt": wtileize(ws["wk"]),
        "wvt": wtileize(ws["wv"]), "wot": wtileize(ws["wo"]),
        "bqt": btile(bs["bq"]), "bkt": btile(bs["bk"]),
        "bvt": btile(bs["bv"]), "bot": btile(bs["bo"]),
        "mask01": mask01, "ident": np.eye(128, dtype=np.float16),
    }
    in_maps = []
    for c in range(NCORES):
        xt_c = f16(xf[c * 1024 : (c + 1) * 1024].T).reshape(16, 128, 1024)
        in_maps.append({"xt": xt_c, **common})
    return in_maps


def assemble(results):
    out = np.empty((B, S, E), np.float32)
    for c in range(NCORES):
        O = results[c]["out"]  # [16 t, 128 p, 1024]; col = u*16 + h
        Oc = O.reshape(E, 64, 16)  # [j, u, h]
        tgt = out[c // 2].reshape(16, 128, E)
        v0 = (c % 2) * 64
        tgt[:, v0 : v0 + 64, :] = Oc.transpose(2, 1, 0)
    return out


def kernel(**inputs):
    global LAST_EXEC_NS
    nc = _get_nc()
    res = run_bass_kernel_spmd(nc, make_in_maps(inputs), core_ids=list(range(NCORES)))
    LAST_EXEC_NS = res.exec_time_ns
    return assemble(res.results)


# revision 29
# speedup vs baseline: 1.0732x; 1.0732x over previous
"""Trainium2 Bass kernel for nn_MultiHeadAttention_44281112822190.

8 NeuronCores, pure data parallelism over the 8192 (b,s) rows: core c takes
rows [c*1024, (c+1)*1024) (batch b = c//2, s-offset (c%2)*1024). No
collectives; the host shards inputs and reassembles the output.

Math notes:
  - The reference applies RoPE to q and k, then contracts q.k at the SAME
    position (per-position head-head attention [B,S,H,H]). RoPE is an
    orthogonal per-position rotation applied identically to q and k, so it
    cancels exactly in the scores: (R q).(R k) = q.k. The kernel skips RoPE
    entirely (freqs inputs are unused).
  - The reference's "h-major flatten" transpose(0,2,1,3).reshape(B,S,-1) is a
    scramble: out[b, h*128 + s//16, (s%16)*128 + d] = att_out[b, s, h, d].
    Each scrambled row draws from 16 consecutive positions of one head, all
    inside one core's shard, so the output projection stays core-local.

Numerics: all matmul operands are fp16 with fp32 PSUM accumulation ->
~7e-4 relative error end-to-end, 1 cycle/row on the PE.

Schedule (v2): the PE streaming floor for the four projections plus
attention is ~464 us/core; this version keeps TensorE dense:
  1. Startup: first wq tile + x^T in 16 fine chunks DMA'd first, weights
     pre-tiled on host so every weight DMA is contiguous (8 KB/partition
     runs). First matmul issues ~5 us in.
  2. Q, K projections: stationary = pre-tiled weight chunks, moving =
     x^T chunks; two N=512 matmuls per LDWEIGHTS; bias added during
     PSUM->SBUF drain. Layout [128 d, 1024 s, 16 h].
  3. V projection in two position-half sweeps (wv streamed twice).
     Attention pairs 0-31 (positions 0-511) are software-pipelined into
     sweep 2's matmul stream in three stages: A = scores matmul +
     exp/mask-sum/reciprocal/normalize, B = att+v transposes (PE), C =
     attO matmul + scatter. Stage lag (A->B 3 slots, B->C 2 slots) covers
     the Scalar/Vector/GpSimd chain latency so TensorE never waits.
  4. Output projection from merged attO halves [128 d, 16 sl, 512] with
     N=512 moving operands, in two sweeps; sweep 1 carries attention
     pairs 32-63 the same way. Drains alternate Vector/GpSimd in
     interleaved phases to keep Vector off the critical path.
Host reassembles the scrambled rows into the final [4, 2048, 2048] output.
"""

import os
import sys

sys.path.insert(0, "/opt/trn_rl_repo")

import numpy as np

import concourse.bacc as bacc
import concourse.mybir as mybir
import concourse.tile as tile
from concourse.bass_utils import run_bass_kernel_spmd

F32 = mybir.dt.float32
F16 = mybir.dt.float16
AF = mybir.ActivationFunctionType
ALU = mybir.AluOpType

B, S, E, H, D = 4, 2048, 2048, 16, 128
NCORES = 8
SCALE = 1.0 / float(np.sqrt(D))

_CACHE = {}
LAST_EXEC_NS = None


def _build():
    nc = bacc.Bacc(trn_type="TRN2", target_bir_lowering=False)

    xt = nc.dram_tensor("xt", [16, 128, 1024], F16, kind="ExternalInput")
    wqt = nc.dram_tensor("wqt", [8, 128, 16, 256], F16, kind="ExternalInput")
    wkt = nc.dram_tensor("wkt", [8, 128, 16, 256], F16, kind="ExternalInput")
    wvt = nc.dram_tensor("wvt", [8, 128, 16, 256], F16, kind="ExternalInput")
    wot = nc.dram_tensor("wot", [8, 128, 16, 256], F16, kind="ExternalInput")
    bqt = nc.dram_tensor("bqt", [128, 16], F32, kind="ExternalInput")
    bkt = nc.dram_tensor("bkt", [128, 16], F32, kind="ExternalInput")
    bvt = nc.dram_tensor("bvt", [128, 16], F32, kind="ExternalInput")
    bot = nc.dram_tensor("bot", [128, 16], F32, kind="ExternalInput")
    mask01 = nc.dram_tensor("mask01", [128, 128], F32, kind="ExternalInput")
    ident = nc.dram_tensor("ident", [128, 128], F16, kind="ExternalInput")
    out = nc.dram_tensor("out", [16, 128, 1024], F32, kind="ExternalOutput")

    with tile.TileContext(nc) as tc:
        with (
            tc.tile_pool(name="const", bufs=1) as cp,
            tc.tile_pool(name="xp", bufs=1) as xp,
            tc.tile_pool(name="qkv", bufs=1) as qkvp,
            tc.tile_pool(name="aop", bufs=1) as aop,
            tc.tile_pool(name="wp", bufs=3) as wp,
            tc.tile_pool(name="gp", bufs=5) as gp,
            tc.tile_pool(name="op", bufs=3) as op,
            tc.tile_pool(name="pp", bufs=4, space="PSUM") as pp,
            tc.tile_pool(name="pa", bufs=2, space="PSUM") as pa,
            tc.tile_pool(name="pb", bufs=1, space="PSUM") as pb,
        ):
            # first weight tile + x first-halves first: the V-A sweep needs
            # only wv[0] + 2 MB of x, so compute starts ~4 us in
            wv_first = wp.tile([128, 16, 256], F16, tag="w")
            nc.sync.dma_start(wv_first[:], wvt[0, :, :, :])
            xa, xb = [], []
            for k in range(16):
                xc = xp.tile([128, 512], F16, tag=f"xa{k}", name=f"xa{k}")
                nc.sync.dma_start(xc[:], xt[k, :, 0:512])
                xa.append(xc)

            mask_sb = cp.tile([128, 128], F32, tag="mask")
            id_sb = cp.tile([128, 128], F16, tag="id")
            nc.sync.dma_start(mask_sb[:], mask01[:, :])
            nc.sync.dma_start(id_sb[:], ident[:, :])
            bias_sb = {}
            for name, t_ in (("bq", bqt), ("bk", bkt), ("bv", bvt), ("bo", bot)):
                b_sb = cp.tile([128, 16], F32, tag=name)
                nc.sync.dma_start(b_sb[:], t_[:, :])
                bias_sb[name] = b_sb

            # xb tiles created here; their DMAs are issued inside the V-A
            # loop so they don't contend with the wv tile fetches V-A needs
            for k in range(16):
                xc = xp.tile([128, 512], F16, tag=f"xb{k}", name=f"xb{k}")
                xb.append(xc)

            qb = qkvp.tile([128, 1024, 16], F16, tag="qb")
            kb = qkvp.tile([128, 1024, 16], F16, tag="kb")
            vbA = qkvp.tile([128, 512, 16], F16, tag="vbA")
            vbB = qkvp.tile([128, 512, 16], F16, tag="vbB")
            attO = [
                aop.tile([128, 16, 512], F16, tag=f"attO{i}", name=f"attO{i}")
                for i in range(2)
            ]

            def drain(dst, ps, bias, t, eng):
                # GpSimd cannot read PSUM, so all drains go through Vector.
                nc.vector.tensor_scalar_add(dst, ps, bias_sb[bias][:, t : t + 1])

            def drain4(dst4, ps, bias, t):
                # chunked drain: keeps long Vector ops from delaying the
                # attention softmax chain during interleaved phases
                b_ap = bias_sb[bias][:, t : t + 1]
                for i in range(4):
                    nc.vector.tensor_scalar_add(
                        dst4[i], ps[:, 128 * i : 128 * i + 128], b_ap
                    )

            # --- attention stages -------------------------------------
            astate = {}

            def attn_A(p):
                G = 2 * p
                sc = pa.tile([128, 256], F32, tag="sc", bufs=2)
                for j in range(2):
                    s0 = (G + j) * 8
                    nc.tensor.matmul(
                        sc[:, 128 * j : 128 * j + 128],
                        qb[:, s0 : s0 + 8, :],
                        kb[:, s0 : s0 + 8, :],
                        start=True, stop=True,
                    )
                e2 = gp.tile([128, 256], F32, tag="e2")
                nc.scalar.activation(e2[:], sc[:, 0:256], AF.Exp, scale=SCALE)
                em2 = e2[:].rearrange("p (g c) -> p g c", g=2)
                den2 = gp.tile([128, 2], F32, tag="den2")
                for j, eng in ((0, nc.vector), (1, nc.vector)):
                    eng.scalar_tensor_tensor(
                        em2[:, j, :], e2[:, 128 * j : 128 * j + 128], 1.0,
                        mask_sb[:], ALU.bypass, ALU.mult,
                        accum_out=den2[:, j : j + 1],
                    )
                rec2 = gp.tile([128, 2], F32, tag="rec2")
                nc.vector.reciprocal(rec2[:], den2[:])
                att2 = gp.tile([128, 2, 128], F16, tag="att2")
                nc.gpsimd.tensor_tensor(
                    att2[:], em2,
                    rec2[:].unsqueeze(2).to_broadcast([128, 2, 128]),
                    ALU.mult,
                )
                astate[p] = {"att2": att2}

            def attn_B(p):
                st = astate[p]
                G = 2 * p
                vb_t, off = (vbA, 0) if p < 32 else (vbB, 512)
                tr = pb.tile([128, 512], F16, tag="tr")
                for j in range(2):
                    s0 = (G + j) * 8 - off
                    nc.tensor.transpose(
                        tr[:, 128 * j : 128 * j + 128], st["att2"][:, j, :],
                        id_sb[:],
                    )
                    nc.tensor.transpose(
                        tr[:, 256 + 128 * j : 384 + 128 * j],
                        vb_t[:, s0 : s0 + 8, :], id_sb[:],
                    )
                trsb = gp.tile([128, 512], F16, tag="trsb")
                nc.scalar.activation(trsb[:], tr[:], AF.Copy)
                st["trsb"] = trsb

            def attn_C(p):
                st = astate.pop(p)
                trsb = st["trsb"]
                ao = pa.tile([128, 256], F32, tag="ao", bufs=1)
                for j in range(2):
                    nc.tensor.matmul(
                        ao[:, 128 * j : 128 * j + 128],
                        trsb[:, 256 + 128 * j : 384 + 128 * j],
                        trsb[:, 128 * j : 128 * j + 128],
                        start=True, stop=True,
                    )
                hh, u2 = p // 32, p % 32
                dst = attO[hh][:].rearrange(
                    "p (g2 i) (u h) -> p g2 i u h", g2=2, h=16
                )[:, :, :, u2, :]
                src = ao[:, 0:256].rearrange("p (g2 i h) -> p g2 i h", g2=2, i=8)
                nc.scalar.activation(dst, src, AF.Copy)

            # pipelined scheduler: one pump() per matmul slot (~213 ns).
            # Stage lags cover the cross-engine softmax chain latency so the
            # TensorE instructions never reach the head of the queue before
            # their operands are ready; spacing-2 keeps the small PSUM rings
            # (sc=2, ao=1, tr=1) from stalling back-to-back stages.
            pipe = {"slot": 0, "a": 0, "b": 0, "c": 0,
                    "a_slot": {}, "b_slot": {}, "last": {"a": -9, "b": -9, "c": -9},
                    "pairs": []}

            def feed(pairs):
                pipe["pairs"].extend(pairs)

            def pump():
                s = pipe["slot"]
                P = pipe["pairs"]
                last = pipe["last"]
                if pipe["c"] < pipe["b"]:
                    p = P[pipe["c"]]
                    if pipe["b_slot"][p] <= s - 5 and last["c"] <= s - 2:
                        attn_C(p)
                        last["c"] = s
                        pipe["c"] += 1
                if pipe["b"] < pipe["a"] and pipe["b"] - pipe["c"] < 5:
                    p = P[pipe["b"]]
                    if pipe["a_slot"][p] <= s - 16 and last["b"] <= s - 2:
                        attn_B(p)
                        pipe["b_slot"][p] = s
                        last["b"] = s
                        pipe["b"] += 1
                if (
                    pipe["a"] < len(P)
                    and pipe["a"] - pipe["b"] < 5
                    and last["a"] <= s - 2
                ):
                    p = P[pipe["a"]]
                    attn_A(p)
                    pipe["a_slot"][p] = s
                    last["a"] = s
                    pipe["a"] += 1
                pipe["slot"] += 1

            def drain_until(n):
                # hard barrier: all attO writers up to pair n issued before
                # any consumer of their columns is enqueued
                while pipe["c"] < n:
                    pump()

            # --- V sweep A: positions 0-511, dense, runs first (needs only
            # wv[0] + the x first-halves, so compute starts early) ---
            for t2 in range(8):
                if t2 == 0:
                    wtile = wv_first
                else:
                    wtile = wp.tile([128, 16, 256], F16, tag="w")
                    nc.sync.dma_start(wtile[:], wvt[t2, :, :, :])
                for k in (2 * t2, 2 * t2 + 1):
                    nc.sync.dma_start(xb[k][:], xt[k, :, 512:1024])
                for half in range(2):
                    t = 2 * t2 + half
                    ps = pp.tile([128, 512], F32, tag="pp")
                    for k in range(16):
                        nc.tensor.matmul(
                            ps[:], wtile[:, k, half * 128 : half * 128 + 128],
                            xa[k][:],
                            start=(k == 0), stop=(k == 15),
                        )
                    drain(vbA[:, :, t], ps[:], "bv", t, "v")

            # --- Q, K projections -> [128 d, 1024 s, 16 h] fp16 ---
            for wdram, bias, dst in ((wqt, "bq", qb), (wkt, "bk", kb)):
                for t2 in range(8):
                    wtile = wp.tile([128, 16, 256], F16, tag="w")
                    nc.sync.dma_start(wtile[:], wdram[t2, :, :, :])
                    for half in range(2):
                        t = 2 * t2 + half
                        psA = pp.tile([128, 512], F32, tag="pp")
                        psB = pp.tile([128, 512], F32, tag="pp")
                        for k in range(16):
                            w_ap = wtile[:, k, half * 128 : half * 128 + 128]
                            nc.tensor.matmul(
                                psA[:], w_ap, xa[k][:],
                                start=(k == 0), stop=(k == 15),
                            )
                            nc.tensor.matmul(
                                psB[:], w_ap, xb[k][:],
                                start=(k == 0), stop=(k == 15),
                            )
                        drain(dst[:, 0:512, t], psA[:], bias, t, "v")
                        drain(dst[:, 512:1024, t], psB[:], bias, t, "v")

            # --- V sweep B: positions 512-1023, carries attn pairs 0-31 ---
            feed(range(0, 32))
            for t2 in range(8):
                wtile = wp.tile([128, 16, 256], F16, tag="w")
                nc.sync.dma_start(wtile[:], wvt[t2, :, :, :])
                for half in range(2):
                    t = 2 * t2 + half
                    ps = pp.tile([128, 512], F32, tag="pp")
                    for k in range(16):
                        nc.tensor.matmul(
                            ps[:],
                            wtile[:, k, half * 128 : half * 128 + 128],
                            xb[k][:],
                            start=(k == 0), stop=(k == 15),
                        )
                        pump()
                    drain4(
                        [vbB[:, 128 * i : 128 * i + 128, t] for i in range(4)],
                        ps[:], "bv", t,
                    )

            # --- output projection, two sweeps over attO halves ---
            def fin_sweep(hh):
                drain_until(32 * hh + 32)
                if hh == 0:
                    feed(range(32, 64))
                for t2 in range(8):
                    wtile = wp.tile([128, 16, 256], F16, tag="w")
                    nc.sync.dma_start(wtile[:], wot[t2, :, :, :])
                    for half in range(2):
                        t = 2 * t2 + half
                        ps = pp.tile([128, 512], F32, tag="pp")
                        last = hh == 1 and t2 == 7 and half == 1
                        if last:
                            # column-split the final group so its drain and
                            # output DMA overlap the second half's matmuls
                            for colh in range(2):
                                c0 = 256 * colh
                                for sl in range(16):
                                    nc.tensor.matmul(
                                        ps[:, c0 : c0 + 256],
                                        wtile[:, sl, half * 128 : half * 128 + 128],
                                        attO[hh][:, sl, c0 : c0 + 256],
                                        start=(sl == 0), stop=(sl == 15),
                                    )
                                ob = op.tile([128, 256], F32, tag="obh")
                                drain(ob[:], ps[:, c0 : c0 + 256], "bo", t, "v")
                                nc.sync.dma_start(
                                    out[t, :, hh * 512 + c0 : hh * 512 + c0 + 256],
                                    ob[:],
                                )
                            continue
                        for sl in range(16):
                            nc.tensor.matmul(
                                ps[:],
                                wtile[:, sl, half * 128 : half * 128 + 128],
                                attO[hh][:, sl, :],
                                start=(sl == 0), stop=(sl == 15),
                            )
                            pump()
                        ob = op.tile([128, 512], F32, tag="ob")
                        if hh == 0:
                            drain4(
                                [ob[:, 128 * i : 128 * i + 128] for i in range(4)],
                                ps[:], "bo", t,
                            )
                        else:
                            drain(ob[:], ps[:], "bo", t, "v")
                        nc.sync.dma_start(
                            out[t, :, hh * 512 : hh * 512 + 512], ob[:]
                        )

            fin_sweep(0)
            drain_until(64)
            fin_sweep(1)

    nc.compile()
    return nc


def _get_nc():
    if "nc" not in _CACHE:
        _CACHE["nc"] = _build()
    return _CACHE["nc"]


def make_in_maps(inputs):
    x = np.ascontiguousarray(np.asarray(inputs["x"], dtype=np.float32))
    ws = {k: np.asarray(inputs[k], dtype=np.float32) for k in ("wq", "wk", "wv", "wo")}
    bs = {k: np.asarray(inputs[k], dtype=np.float32) for k in ("bq", "bk", "bv", "bo")}

    xf = x.reshape(B * S, E)
    f16 = lambda a: np.ascontiguousarray(a).astype(np.float16)

    def wtileize(w):
        # [t2, p, k, c] tiles: element (p, k, c) = w.T[k*128+p, t2*256+c]
        return np.ascontiguousarray(
            w.T.astype(np.float16).reshape(16, 128, 8, 256).transpose(2, 1, 0, 3)
        )

    btile = lambda b: np.ascontiguousarray(b.reshape(16, 128).T)
    ii = np.arange(128) // 16
    mask01 = (ii[:, None] == ii[None, :]).astype(np.float32)
    common = {
        "wqt": wtileize(ws["wq"]), "wkt": wtileize(ws["wk"]),
        "wvt": wtileize(ws["wv"]), "wot": wtileize(ws["wo"]),
        "bqt": btile(bs["bq"]), "bkt": btile(bs["bk"]),
        "bvt": btile(bs["bv"]), "bot": btile(bs["bo"]),
        "mask01": mask01, "ident": np.eye(128, dtype=np.float16),
    }
    in_maps = []
    for c in range(NCORES):
        xt_c = f16(xf[c * 1024 : (c + 1) * 1024].T).reshape(16, 128, 1024)
        in_maps.append({"xt": xt_c, **common})
    return in_maps


def assemble(results):
    out = np.empty((B, S, E), np.float32)
    for c in range(NCORES):
        O = results[c]["out"]  # [16 t, 128 p, 1024]; col = u*16 + h
        Oc = O.reshape(E, 64, 16)  # [j, u, h]
        tgt = out[c // 2].reshape(16, 128, E)
        v0 = (c % 2) * 64
        tgt[:, v0 : v0 + 64, :] = Oc.transpose(2, 1, 0)
    return out


def kernel(**inputs):
    global LAST_EXEC_NS
    nc = _get_nc()
    res = run_bass_kernel_spmd(nc, make_in_maps(inputs), core_ids=list(range(NCORES)))
    LAST_EXEC_NS = res.exec_time_ns
    return assemble(res.results)


# revision 31
# speedup vs baseline: 1.0762x; 1.0028x over previous
"""Trainium2 Bass kernel for nn_MultiHeadAttention_44281112822190.

8 NeuronCores, pure data parallelism over the 8192 (b,s) rows: core c takes
rows [c*1024, (c+1)*1024) (batch b = c//2, s-offset (c%2)*1024). No
collectives; the host shards inputs and reassembles the output.

Math notes:
  - The reference applies RoPE to q and k, then contracts q.k at the SAME
    position (per-position head-head attention [B,S,H,H]). RoPE is an
    orthogonal per-position rotation applied identically to q and k, so it
    cancels exactly in the scores: (R q).(R k) = q.k. The kernel skips RoPE
    entirely (freqs inputs are unused).
  - The reference's "h-major flatten" transpose(0,2,1,3).reshape(B,S,-1) is a
    scramble: out[b, h*128 + s//16, (s%16)*128 + d] = att_out[b, s, h, d].
    Each scrambled row draws from 16 consecutive positions of one head, all
    inside one core's shard, so the output projection stays core-local.

Numerics: all matmul operands are fp16 with fp32 PSUM accumulation ->
~7e-4 relative error end-to-end, 1 cycle/row on the PE.

Schedule (v2): the PE streaming floor for the four projections plus
attention is ~464 us/core; this version keeps TensorE dense:
  1. Startup: first wq tile + x^T in 16 fine chunks DMA'd first, weights
     pre-tiled on host so every weight DMA is contiguous (8 KB/partition
     runs). First matmul issues ~5 us in.
  2. Q, K projections: stationary = pre-tiled weight chunks, moving =
     x^T chunks; two N=512 matmuls per LDWEIGHTS; bias added during
     PSUM->SBUF drain. Layout [128 d, 1024 s, 16 h].
  3. V projection in two position-half sweeps (wv streamed twice).
     Attention pairs 0-31 (positions 0-511) are software-pipelined into
     sweep 2's matmul stream in three stages: A = scores matmul +
     exp/mask-sum/reciprocal/normalize, B = att+v transposes (PE), C =
     attO matmul + scatter. Stage lag (A->B 3 slots, B->C 2 slots) covers
     the Scalar/Vector/GpSimd chain latency so TensorE never waits.
  4. Output projection from merged attO halves [128 d, 16 sl, 512] with
     N=512 moving operands, in two sweeps; sweep 1 carries attention
     pairs 32-63 the same way. Drains alternate Vector/GpSimd in
     interleaved phases to keep Vector off the critical path.
Host reassembles the scrambled rows into the final [4, 2048, 2048] output.
"""

import os
import sys

sys.path.insert(0, "/opt/trn_rl_repo")

import numpy as np

import concourse.bacc as bacc
import concourse.mybir as mybir
import concourse.tile as tile
from concourse.bass_utils import run_bass_kernel_spmd

F32 = mybir.dt.float32
F16 = mybir.dt.float16
AF = mybir.ActivationFunctionType
ALU = mybir.AluOpType

B, S, E, H, D = 4, 2048, 2048, 16, 128
NCORES = 8
SCALE = 1.0 / float(np.sqrt(D))

_CACHE = {}
LAST_EXEC_NS = None


def _build():
    nc = bacc.Bacc(trn_type="TRN2", target_bir_lowering=False)

    xt = nc.dram_tensor("xt", [16, 128, 1024], F16, kind="ExternalInput")
    wqt = nc.dram_tensor("wqt", [8, 128, 16, 256], F16, kind="ExternalInput")
    wkt = nc.dram_tensor("wkt", [8, 128, 16, 256], F16, kind="ExternalInput")
    wvt = nc.dram_tensor("wvt", [8, 128, 16, 256], F16, kind="ExternalInput")
    wot = nc.dram_tensor("wot", [8, 128, 16, 256], F16, kind="ExternalInput")
    bqt = nc.dram_tensor("bqt", [128, 16], F32, kind="ExternalInput")
    bkt = nc.dram_tensor("bkt", [128, 16], F32, kind="ExternalInput")
    bvt = nc.dram_tensor("bvt", [128, 16], F32, kind="ExternalInput")
    bot = nc.dram_tensor("bot", [128, 16], F32, kind="ExternalInput")
    mask01 = nc.dram_tensor("mask01", [128, 128], F32, kind="ExternalInput")
    ident = nc.dram_tensor("ident", [128, 128], F16, kind="ExternalInput")
    out = nc.dram_tensor("out", [16, 128, 1024], F32, kind="ExternalOutput")

    with tile.TileContext(nc) as tc:
        with (
            tc.tile_pool(name="const", bufs=1) as cp,
            tc.tile_pool(name="xp", bufs=1) as xp,
            tc.tile_pool(name="qkv", bufs=1) as qkvp,
            tc.tile_pool(name="aop", bufs=1) as aop,
            tc.tile_pool(name="wp", bufs=3) as wp,
            tc.tile_pool(name="gp", bufs=5) as gp,
            tc.tile_pool(name="op", bufs=3) as op,
            tc.tile_pool(name="pp", bufs=4, space="PSUM") as pp,
            tc.tile_pool(name="pa", bufs=2, space="PSUM") as pa,
            tc.tile_pool(name="pb", bufs=1, space="PSUM") as pb,
        ):
            # first weight tile + x first-halves first: the V-A sweep needs
            # only wv[0] + 2 MB of x, so compute starts ~4 us in. wv[0] is
            # fetched in 4 pieces so its first k-chunks land ASAP.
            wv_first = wp.tile([128, 16, 256], F16, tag="w")
            for q in range(4):
                nc.sync.dma_start(
                    wv_first[:, 4 * q : 4 * q + 4, :], wvt[0, :, 4 * q : 4 * q + 4, :]
                )
            xa, xb = [], []
            for k in range(16):
                xc = xp.tile([128, 512], F16, tag=f"xa{k}", name=f"xa{k}")
                nc.sync.dma_start(xc[:], xt[k, :, 0:512])
                xa.append(xc)

            bias_sb = {}
            for name, t_ in (("bq", bqt), ("bk", bkt), ("bv", bvt), ("bo", bot)):
                b_sb = cp.tile([128, 16], F32, tag=name)
                nc.sync.dma_start(b_sb[:], t_[:, :])
                bias_sb[name] = b_sb
            mask_sb = cp.tile([128, 128], F32, tag="mask")
            id_sb = cp.tile([128, 128], F16, tag="id")

            # xb tiles created here; their DMAs are issued inside the V-A
            # loop so they don't contend with the wv tile fetches V-A needs
            for k in range(16):
                xc = xp.tile([128, 512], F16, tag=f"xb{k}", name=f"xb{k}")
                xb.append(xc)

            qb = qkvp.tile([128, 1024, 16], F16, tag="qb")
            kb = qkvp.tile([128, 1024, 16], F16, tag="kb")
            vbA = qkvp.tile([128, 512, 16], F16, tag="vbA")
            vbB = qkvp.tile([128, 512, 16], F16, tag="vbB")
            attO = [
                aop.tile([128, 16, 512], F16, tag=f"attO{i}", name=f"attO{i}")
                for i in range(2)
            ]

            def drain(dst, ps, bias, t, eng):
                # GpSimd cannot read PSUM, so all drains go through Vector.
                nc.vector.tensor_scalar_add(dst, ps, bias_sb[bias][:, t : t + 1])

            def drain4(dst4, ps, bias, t):
                # chunked drain: keeps long Vector ops from delaying the
                # attention softmax chain during interleaved phases
                b_ap = bias_sb[bias][:, t : t + 1]
                for i in range(4):
                    nc.vector.tensor_scalar_add(
                        dst4[i], ps[:, 128 * i : 128 * i + 128], b_ap
                    )

            # --- attention stages -------------------------------------
            astate = {}

            def attn_A(p):
                G = 2 * p
                sc = pa.tile([128, 256], F32, tag="sc", bufs=2)
                for j in range(2):
                    s0 = (G + j) * 8
                    nc.tensor.matmul(
                        sc[:, 128 * j : 128 * j + 128],
                        qb[:, s0 : s0 + 8, :],
                        kb[:, s0 : s0 + 8, :],
                        start=True, stop=True,
                    )
                e2 = gp.tile([128, 256], F32, tag="e2")
                nc.scalar.activation(e2[:], sc[:, 0:256], AF.Exp, scale=SCALE)
                em2 = e2[:].rearrange("p (g c) -> p g c", g=2)
                den2 = gp.tile([128, 2], F32, tag="den2")
                for j, eng in ((0, nc.vector), (1, nc.vector)):
                    eng.scalar_tensor_tensor(
                        em2[:, j, :], e2[:, 128 * j : 128 * j + 128], 1.0,
                        mask_sb[:], ALU.bypass, ALU.mult,
                        accum_out=den2[:, j : j + 1],
                    )
                rec2 = gp.tile([128, 2], F32, tag="rec2")
                nc.vector.reciprocal(rec2[:], den2[:])
                att2 = gp.tile([128, 2, 128], F16, tag="att2")
                nc.gpsimd.tensor_tensor(
                    att2[:], em2,
                    rec2[:].unsqueeze(2).to_broadcast([128, 2, 128]),
                    ALU.mult,
                )
                astate[p] = {"att2": att2}

            def attn_B(p):
                st = astate[p]
                G = 2 * p
                vb_t, off = (vbA, 0) if p < 32 else (vbB, 512)
                tr = pb.tile([128, 512], F16, tag="tr")
                for j in range(2):
                    s0 = (G + j) * 8 - off
                    nc.tensor.transpose(
                        tr[:, 128 * j : 128 * j + 128], st["att2"][:, j, :],
                        id_sb[:],
                    )
                    nc.tensor.transpose(
                        tr[:, 256 + 128 * j : 384 + 128 * j],
                        vb_t[:, s0 : s0 + 8, :], id_sb[:],
                    )
                trsb = gp.tile([128, 512], F16, tag="trsb")
                nc.scalar.activation(trsb[:], tr[:], AF.Copy)
                st["trsb"] = trsb

            def attn_C(p):
                st = astate.pop(p)
                trsb = st["trsb"]
                ao = pa.tile([128, 256], F32, tag="ao", bufs=1)
                for j in range(2):
                    nc.tensor.matmul(
                        ao[:, 128 * j : 128 * j + 128],
                        trsb[:, 256 + 128 * j : 384 + 128 * j],
                        trsb[:, 128 * j : 128 * j + 128],
                        start=True, stop=True,
                    )
                hh, u2 = p // 32, p % 32
                dst = attO[hh][:].rearrange(
                    "p (g2 i) (u h) -> p g2 i u h", g2=2, h=16
                )[:, :, :, u2, :]
                src = ao[:, 0:256].rearrange("p (g2 i h) -> p g2 i h", g2=2, i=8)
                nc.scalar.activation(dst, src, AF.Copy)

            # pipelined scheduler: one pump() per matmul slot (~213 ns).
            # Stage lags cover the cross-engine softmax chain latency so the
            # TensorE instructions never reach the head of the queue before
            # their operands are ready; spacing-2 keeps the small PSUM rings
            # (sc=2, ao=1, tr=1) from stalling back-to-back stages.
            pipe = {"slot": 0, "a": 0, "b": 0, "c": 0,
                    "a_slot": {}, "b_slot": {}, "last": {"a": -9, "b": -9, "c": -9},
                    "pairs": []}

            def feed(pairs):
                pipe["pairs"].extend(pairs)

            def pump():
                s = pipe["slot"]
                P = pipe["pairs"]
                last = pipe["last"]
                if pipe["c"] < pipe["b"]:
                    p = P[pipe["c"]]
                    if pipe["b_slot"][p] <= s - 5 and last["c"] <= s - 2:
                        attn_C(p)
                        last["c"] = s
                        pipe["c"] += 1
                if pipe["b"] < pipe["a"] and pipe["b"] - pipe["c"] < 5:
                    p = P[pipe["b"]]
                    if pipe["a_slot"][p] <= s - 16 and last["b"] <= s - 2:
                        attn_B(p)
                        pipe["b_slot"][p] = s
                        last["b"] = s
                        pipe["b"] += 1
                if (
                    pipe["a"] < len(P)
                    and pipe["a"] - pipe["b"] < 5
                    and last["a"] <= s - 2
                ):
                    p = P[pipe["a"]]
                    attn_A(p)
                    pipe["a_slot"][p] = s
                    last["a"] = s
                    pipe["a"] += 1
                pipe["slot"] += 1

            def drain_until(n):
                # hard barrier: all attO writers up to pair n issued before
                # any consumer of their columns is enqueued
                while pipe["c"] < n:
                    pump()

            # --- V sweep A: positions 0-511, dense, runs first (needs only
            # wv[0] + the x first-halves, so compute starts early) ---
            for t2 in range(8):
                if t2 == 0:
                    wtile = wv_first
                else:
                    wtile = wp.tile([128, 16, 256], F16, tag="w")
                    nc.sync.dma_start(wtile[:], wvt[t2, :, :, :])
                if t2 == 2:
                    # mask/ident aren't needed until attention (~280 us in)
                    nc.sync.dma_start(mask_sb[:], mask01[:, :])
                    nc.sync.dma_start(id_sb[:], ident[:, :])
                if 3 <= t2 <= 6:
                    for k in range(4 * (t2 - 3), 4 * (t2 - 3) + 4):
                        nc.sync.dma_start(xb[k][:], xt[k, :, 512:1024])
                for half in range(2):
                    t = 2 * t2 + half
                    ps = pp.tile([128, 512], F32, tag="pp")
                    for k in range(16):
                        nc.tensor.matmul(
                            ps[:], wtile[:, k, half * 128 : half * 128 + 128],
                            xa[k][:],
                            start=(k == 0), stop=(k == 15),
                        )
                    drain(vbA[:, :, t], ps[:], "bv", t, "v")

            # --- Q, K projections -> [128 d, 1024 s, 16 h] fp16 ---
            for wdram, bias, dst in ((wqt, "bq", qb), (wkt, "bk", kb)):
                for t2 in range(8):
                    wtile = wp.tile([128, 16, 256], F16, tag="w")
                    nc.sync.dma_start(wtile[:], wdram[t2, :, :, :])
                    for half in range(2):
                        t = 2 * t2 + half
                        psA = pp.tile([128, 512], F32, tag="pp")
                        psB = pp.tile([128, 512], F32, tag="pp")
                        for k in range(16):
                            w_ap = wtile[:, k, half * 128 : half * 128 + 128]
                            nc.tensor.matmul(
                                psA[:], w_ap, xa[k][:],
                                start=(k == 0), stop=(k == 15),
                            )
                            nc.tensor.matmul(
                                psB[:], w_ap, xb[k][:],
                                start=(k == 0), stop=(k == 15),
                            )
                        drain(dst[:, 0:512, t], psA[:], bias, t, "v")
                        drain(dst[:, 512:1024, t], psB[:], bias, t, "v")

            # --- V sweep B: positions 512-1023, carries attn pairs 0-31 ---
            feed(range(0, 32))
            for t2 in range(8):
                wtile = wp.tile([128, 16, 256], F16, tag="w")
                nc.sync.dma_start(wtile[:], wvt[t2, :, :, :])
                for half in range(2):
                    t = 2 * t2 + half
                    ps = pp.tile([128, 512], F32, tag="pp")
                    for k in range(16):
                        nc.tensor.matmul(
                            ps[:],
                            wtile[:, k, half * 128 : half * 128 + 128],
                            xb[k][:],
                            start=(k == 0), stop=(k == 15),
                        )
                        pump()
                    drain4(
                        [vbB[:, 128 * i : 128 * i + 128, t] for i in range(4)],
                        ps[:], "bv", t,
                    )

            # --- output projection, two sweeps over attO halves ---
            def fin_sweep(hh):
                drain_until(32 * hh + 32)
                if hh == 0:
                    feed(range(32, 64))
                for t2 in range(8):
                    wtile = wp.tile([128, 16, 256], F16, tag="w")
                    nc.sync.dma_start(wtile[:], wot[t2, :, :, :])
                    for half in range(2):
                        t = 2 * t2 + half
                        ps = pp.tile([128, 512], F32, tag="pp")
                        last = hh == 1 and t2 == 7 and half == 1
                        if last:
                            # column-split the final group so its drain and
                            # output DMA overlap the second half's matmuls
                            for colh in range(2):
                                c0 = 256 * colh
                                for sl in range(16):
                                    nc.tensor.matmul(
                                        ps[:, c0 : c0 + 256],
                                        wtile[:, sl, half * 128 : half * 128 + 128],
                                        attO[hh][:, sl, c0 : c0 + 256],
                                        start=(sl == 0), stop=(sl == 15),
                                    )
                                ob = op.tile([128, 256], F32, tag="obh")
                                drain(ob[:], ps[:, c0 : c0 + 256], "bo", t, "v")
                                nc.sync.dma_start(
                                    out[t, :, hh * 512 + c0 : hh * 512 + c0 + 256],
                                    ob[:],
                                )
                            continue
                        for sl in range(16):
                            nc.tensor.matmul(
                                ps[:],
                                wtile[:, sl, half * 128 : half * 128 + 128],
                                attO[hh][:, sl, :],
                                start=(sl == 0), stop=(sl == 15),
                            )
                            pump()
                        ob = op.tile([128, 512], F32, tag="ob")
                        if hh == 0:
                            drain4(
                                [ob[:, 128 * i : 128 * i + 128] for i in range(4)],
                                ps[:], "bo", t,
                            )
                        else:
                            drain(ob[:], ps[:], "bo", t, "v")
                        nc.sync.dma_start(
                            out[t, :, hh * 512 : hh * 512 + 512], ob[:]
                        )

            fin_sweep(0)
            drain_until(64)
            fin_sweep(1)

    nc.compile()
    return nc


def _get_nc():
    if "nc" not in _CACHE:
        _CACHE["nc"] = _build()
    return _CACHE["nc"]


def make_in_maps(inputs):
    x = np.ascontiguousarray(np.asarray(inputs["x"], dtype=np.float32))
    ws = {k: np.asarray(inputs[k], dtype=np.float32) for k in ("wq", "wk", "wv", "wo")}
    bs = {k: np.asarray(inputs[k], dtype=np.float32) for k in ("bq", "bk", "bv", "bo")}

    xf = x.reshape(B * S, E)
    f16 = lambda a: np.ascontiguousarray(a).astype(np.float16)

    def wtileize(w):
        # [t2, p, k, c] tiles: element (p, k, c) = w.T[k*128+p, t2*256+c]
        return np.ascontiguousarray(
            w.T.astype(np.float16).reshape(16, 128, 8, 256).transpose(2, 1, 0, 3)
        )

    btile = lambda b: np.ascontiguousarray(b.reshape(16, 128).T)
    ii = np.arange(128) // 16
    mask01 = (ii[:, None] == ii[None, :]).astype(np.float32)
    common = {
        "wqt": wtileize(ws["wq"]), "wkt": wtileize(ws["wk"]),
        "wvt": wtileize(ws["wv"]), "wot": wtileize(ws["wo"]),
        "bqt": btile(bs["bq"]), "bkt": btile(bs["bk"]),
        "bvt": btile(bs["bv"]), "bot": btile(bs["bo"]),
        "mask01": mask01, "ident": np.eye(128, dtype=np.float16),
    }
    in_maps = []
    for c in range(NCORES):
        xt_c = f16(xf[c * 1024 : (c + 1) * 1024].T).reshape(16, 128, 1024)
        in_maps.append({"xt": xt_c, **common})
    return in_maps


def assemble(results):
    out = np.empty((B, S, E), np.float32)
    for c in range(NCORES):
        O = results[c]["out"]  # [16 t, 128 p, 1024]; col = u*16 + h
        Oc = O.reshape(E, 64, 16)  # [j, u, h]
        tgt = out[c // 2].reshape(16, 128, E)
        v0 = (c % 2) * 64
        tgt[:, v0 : v0 + 64, :] = Oc.transpose(2, 1, 0)
    return out


def kernel(**inputs):
    global LAST_EXEC_NS
    nc = _get_nc()
    res = run_bass_kernel_spmd(nc, make_in_maps(inputs), core_ids=list(range(NCORES)))
    LAST_EXEC_NS = res.exec_time_ns
    return assemble(res.results)


# revision 42
# speedup vs baseline: 1.0789x; 1.0024x over previous
"""Trainium2 Bass kernel for nn_MultiHeadAttention_44281112822190.

8 NeuronCores, pure data parallelism over the 8192 (b,s) rows: core c takes
rows [c*1024, (c+1)*1024) (batch b = c//2, s-offset (c%2)*1024). No
collectives; the host shards inputs and reassembles the output.

Math notes:
  - The reference applies RoPE to q and k, then contracts q.k at the SAME
    position (per-position head-head attention [B,S,H,H]). RoPE is an
    orthogonal per-position rotation applied identically to q and k, so it
    cancels exactly in the scores: (R q).(R k) = q.k. The kernel skips RoPE
    entirely (freqs inputs are unused).
  - The reference's "h-major flatten" transpose(0,2,1,3).reshape(B,S,-1) is a
    scramble: out[b, h*128 + s//16, (s%16)*128 + d] = att_out[b, s, h, d].
    Each scrambled row draws from 16 consecutive positions of one head, all
    inside one core's shard, so the output projection stays core-local.

Numerics: all matmul operands are fp16 with fp32 PSUM accumulation ->
~7e-4 relative error end-to-end, 1 cycle/row on the PE.

Schedule (v2): the PE streaming floor for the four projections plus
attention is ~464 us/core; this version keeps TensorE dense:
  1. Startup: first wq tile + x^T in 16 fine chunks DMA'd first, weights
     pre-tiled on host so every weight DMA is contiguous (8 KB/partition
     runs). First matmul issues ~5 us in.
  2. Q, K projections: stationary = pre-tiled weight chunks, moving =
     x^T chunks; two N=512 matmuls per LDWEIGHTS; bias added during
     PSUM->SBUF drain. Layout [128 d, 1024 s, 16 h].
  3. V projection in two position-half sweeps (wv streamed twice).
     Attention pairs 0-31 (positions 0-511) are software-pipelined into
     sweep 2's matmul stream in three stages: A = scores matmul +
     exp/mask-sum/reciprocal/normalize, B = att+v transposes (PE), C =
     attO matmul + scatter. Stage lag (A->B 3 slots, B->C 2 slots) covers
     the Scalar/Vector/GpSimd chain latency so TensorE never waits.
  4. Output projection from merged attO halves [128 d, 16 sl, 512] with
     N=512 moving operands, in two sweeps; sweep 1 carries attention
     pairs 32-63 the same way. Drains alternate Vector/GpSimd in
     interleaved phases to keep Vector off the critical path.
Host reassembles the scrambled rows into the final [4, 2048, 2048] output.
"""

import os
import sys

sys.path.insert(0, "/opt/trn_rl_repo")

import numpy as np

import concourse.bacc as bacc
import concourse.mybir as mybir
import concourse.tile as tile
from concourse.bass_utils import run_bass_kernel_spmd

F32 = mybir.dt.float32
F16 = mybir.dt.float16
AF = mybir.ActivationFunctionType
ALU = mybir.AluOpType

B, S, E, H, D = 4, 2048, 2048, 16, 128
NCORES = 8
SCALE = 1.0 / float(np.sqrt(D))

_CACHE = {}
LAST_EXEC_NS = None


def _build():
    nc = bacc.Bacc(trn_type="TRN2", target_bir_lowering=False)

    xt = nc.dram_tensor("xt", [16, 128, 1024], F16, kind="ExternalInput")
    wqt = nc.dram_tensor("wqt", [8, 128, 16, 256], F16, kind="ExternalInput")
    wkt = nc.dram_tensor("wkt", [8, 128, 16, 256], F16, kind="ExternalInput")
    wvt = nc.dram_tensor("wvt", [8, 128, 16, 256], F16, kind="ExternalInput")
    wot = nc.dram_tensor("wot", [8, 128, 16, 256], F16, kind="ExternalInput")
    bqt = nc.dram_tensor("bqt", [128, 16], F32, kind="ExternalInput")
    bkt = nc.dram_tensor("bkt", [128, 16], F32, kind="ExternalInput")
    bvt = nc.dram_tensor("bvt", [128, 16], F32, kind="ExternalInput")
    bot = nc.dram_tensor("bot", [128, 16], F32, kind="ExternalInput")
    mask01 = nc.dram_tensor("mask01", [128, 128], F32, kind="ExternalInput")
    ident = nc.dram_tensor("ident", [128, 128], F16, kind="ExternalInput")
    out = nc.dram_tensor("out", [16, 128, 1024], F32, kind="ExternalOutput")

    with tile.TileContext(nc) as tc:
        with (
            tc.tile_pool(name="const", bufs=1) as cp,
            tc.tile_pool(name="xp", bufs=1) as xp,
            tc.tile_pool(name="qkv", bufs=1) as qkvp,
            tc.tile_pool(name="aop", bufs=1) as aop,
            tc.tile_pool(name="wp", bufs=3) as wp,
            tc.tile_pool(name="gp", bufs=5) as gp,
            tc.tile_pool(name="op", bufs=3) as op,
            tc.tile_pool(name="pp", bufs=4, space="PSUM") as pp,
            tc.tile_pool(name="pa", bufs=2, space="PSUM") as pa,
            tc.tile_pool(name="pb", bufs=1, space="PSUM") as pb,
        ):
            # first weight tile + x first-halves first: the V-A sweep needs
            # only wv[0] + 2 MB of x, so compute starts ~4 us in. wv[0] is
            # fetched in 4 pieces so its first k-chunks land ASAP.
            wv_first = wp.tile([128, 16, 256], F16, tag="w")
            for q in range(4):
                nc.sync.dma_start(
                    wv_first[:, 4 * q : 4 * q + 4, :], wvt[0, :, 4 * q : 4 * q + 4, :]
                )
            xa, xb = [], []
            for k in range(16):
                xc = xp.tile([128, 512], F16, tag=f"xa{k}", name=f"xa{k}")
                nc.sync.dma_start(xc[:], xt[k, :, 0:512])
                xa.append(xc)

            bias_sb = {}
            for name, t_ in (("bq", bqt), ("bk", bkt), ("bv", bvt), ("bo", bot)):
                b_sb = cp.tile([128, 16], F32, tag=name)
                nc.sync.dma_start(b_sb[:], t_[:, :])
                bias_sb[name] = b_sb
            mask_sb = cp.tile([128, 128], F32, tag="mask")
            id_sb = cp.tile([128, 128], F16, tag="id")

            # xb tiles created here; their DMAs are issued inside the V-A
            # loop so they don't contend with the wv tile fetches V-A needs
            for k in range(16):
                xc = xp.tile([128, 512], F16, tag=f"xb{k}", name=f"xb{k}")
                xb.append(xc)

            qb = qkvp.tile([128, 1024, 16], F16, tag="qb")
            kb = qkvp.tile([128, 1024, 16], F16, tag="kb")
            vbA = qkvp.tile([128, 512, 16], F16, tag="vbA")
            vbB = qkvp.tile([128, 512, 16], F16, tag="vbB")
            attO = [
                aop.tile([128, 16, 512], F16, tag=f"attO{i}", name=f"attO{i}")
                for i in range(2)
            ]

            def drain(dst, ps, bias, t, eng):
                # GpSimd cannot read PSUM, so all drains go through Vector.
                nc.vector.tensor_scalar_add(dst, ps, bias_sb[bias][:, t : t + 1])

            def drain4(dst4, ps, bias, t):
                # chunked drain: keeps long Vector ops from delaying the
                # attention softmax chain during interleaved phases
                b_ap = bias_sb[bias][:, t : t + 1]
                for i in range(4):
                    nc.vector.tensor_scalar_add(
                        dst4[i], ps[:, 128 * i : 128 * i + 128], b_ap
                    )

            # --- attention stages -------------------------------------
            astate = {}

            def attn_A(p):
                G = 2 * p
                sc = pa.tile([128, 256], F32, tag="sc", bufs=2)
                for j in range(2):
                    s0 = (G + j) * 8
                    nc.tensor.matmul(
                        sc[:, 128 * j : 128 * j + 128],
                        qb[:, s0 : s0 + 8, :],
                        kb[:, s0 : s0 + 8, :],
                        start=True, stop=True,
                    )
                e2 = gp.tile([128, 256], F32, tag="e2")
                nc.scalar.activation(e2[:], sc[:, 0:256], AF.Exp, scale=SCALE)
                em2 = e2[:].rearrange("p (g c) -> p g c", g=2)
                den2 = gp.tile([128, 2], F32, tag="den2")
                for j, eng in ((0, nc.vector), (1, nc.vector)):
                    eng.scalar_tensor_tensor(
                        em2[:, j, :], e2[:, 128 * j : 128 * j + 128], 1.0,
                        mask_sb[:], ALU.bypass, ALU.mult,
                        accum_out=den2[:, j : j + 1],
                    )
                rec2 = gp.tile([128, 2], F32, tag="rec2")
                nc.vector.reciprocal(rec2[:], den2[:])
                att2 = gp.tile([128, 2, 128], F16, tag="att2")
                nc.gpsimd.tensor_tensor(
                    att2[:], em2,
                    rec2[:].unsqueeze(2).to_broadcast([128, 2, 128]),
                    ALU.mult,
                )
                astate[p] = {"att2": att2}

            def _b_transposes(p, j, tr):
                st = astate[p]
                vb_t, off = (vbA, 0) if p < 32 else (vbB, 512)
                s0 = (2 * p + j) * 8 - off
                nc.tensor.transpose(
                    tr[:, 128 * j : 128 * j + 128], st["att2"][:, j, :],
                    id_sb[:],
                )
                nc.tensor.transpose(
                    tr[:, 256 + 128 * j : 384 + 128 * j],
                    vb_t[:, s0 : s0 + 8, :], id_sb[:],
                )

            def attn_B1(p):
                tr = pb.tile([128, 512], F16, tag="tr")
                astate[p]["tr"] = tr
                _b_transposes(p, 0, tr)

            def attn_B2(p):
                tr = astate[p]["tr"]
                _b_transposes(p, 1, tr)
                trsb = gp.tile([128, 512], F16, tag="trsb")
                nc.scalar.activation(trsb[:], tr[:], AF.Copy)
                astate[p]["trsb"] = trsb

            def attn_C(p):
                st = astate.pop(p)
                trsb = st["trsb"]
                ao = pa.tile([128, 256], F32, tag="ao", bufs=1)
                for j in range(2):
                    nc.tensor.matmul(
                        ao[:, 128 * j : 128 * j + 128],
                        trsb[:, 256 + 128 * j : 384 + 128 * j],
                        trsb[:, 128 * j : 128 * j + 128],
                        start=True, stop=True,
                    )
                hh, u2 = p // 32, p % 32
                dst = attO[hh][:].rearrange(
                    "p (g2 i) (u h) -> p g2 i u h", g2=2, h=16
                )[:, :, :, u2, :]
                src = ao[:, 0:256].rearrange("p (g2 i h) -> p g2 i h", g2=2, i=8)
                nc.scalar.activation(dst, src, AF.Copy)

            # pipelined scheduler: one pump() per matmul slot (~213 ns).
            # Stage lags cover the cross-engine softmax chain latency so the
            # TensorE instructions never reach the head of the queue before
            # their operands are ready; spacing-2 keeps the small PSUM rings
            # (sc=2, ao=1, tr=1) from stalling back-to-back stages.
            pipe = {"slot": 0, "a": 0, "b1": 0, "b2": 0, "c": 0,
                    "a_slot": {}, "b1_slot": {}, "b2_slot": {},
                    "last": {"a": -9, "b1": -9, "b2": -9, "c": -9},
                    "pairs": []}

            def feed(pairs):
                pipe["pairs"].extend(pairs)

            def pump():
                s = pipe["slot"]
                P = pipe["pairs"]
                last = pipe["last"]
                if pipe["c"] < pipe["b2"]:
                    p = P[pipe["c"]]
                    if pipe["b2_slot"][p] <= s - 5 and last["c"] <= s - 2:
                        attn_C(p)
                        last["c"] = s
                        pipe["c"] += 1
                if pipe["b2"] < pipe["b1"]:
                    p = P[pipe["b2"]]
                    if pipe["b1_slot"][p] <= s - 2 and last["b2"] <= s - 2:
                        attn_B2(p)
                        pipe["b2_slot"][p] = s
                        last["b2"] = s
                        pipe["b2"] += 1
                if pipe["b1"] < pipe["a"] and pipe["b1"] - pipe["c"] < 5:
                    p = P[pipe["b1"]]
                    if pipe["a_slot"][p] <= s - 16 and last["b1"] <= s - 2:
                        attn_B1(p)
                        pipe["b1_slot"][p] = s
                        last["b1"] = s
                        pipe["b1"] += 1
                if (
                    pipe["a"] < len(P)
                    and pipe["a"] - pipe["b2"] < 5
                    and last["a"] <= s - 2
                ):
                    p = P[pipe["a"]]
                    attn_A(p)
                    pipe["a_slot"][p] = s
                    last["a"] = s
                    pipe["a"] += 1
                pipe["slot"] += 1

            def drain_until(n):
                # hard barrier: all attO writers up to pair n issued before
                # any consumer of their columns is enqueued
                while pipe["c"] < n:
                    pump()

            # --- V sweep A: positions 0-511, dense, runs first (needs only
            # wv[0] + the x first-halves, so compute starts early) ---
            for t2 in range(8):
                if t2 == 0:
                    wtile = wv_first
                else:
                    wtile = wp.tile([128, 16, 256], F16, tag="w")
                    nc.sync.dma_start(wtile[:], wvt[t2, :, :, :])
                if t2 == 2:
                    # mask/ident aren't needed until attention (~280 us in)
                    nc.sync.dma_start(mask_sb[:], mask01[:, :])
                    nc.sync.dma_start(id_sb[:], ident[:, :])
                if 3 <= t2 <= 6:
                    for k in range(4 * (t2 - 3), 4 * (t2 - 3) + 4):
                        nc.sync.dma_start(xb[k][:], xt[k, :, 512:1024])
                for half in range(2):
                    t = 2 * t2 + half
                    ps = pp.tile([128, 512], F32, tag="pp")
                    for k in range(16):
                        nc.tensor.matmul(
                            ps[:], wtile[:, k, half * 128 : half * 128 + 128],
                            xa[k][:],
                            start=(k == 0), stop=(k == 15),
                        )
                    drain(vbA[:, :, t], ps[:], "bv", t, "v")

            # --- Q, K projections -> [128 d, 1024 s, 16 h] fp16 ---
            for wdram, bias, dst in ((wqt, "bq", qb), (wkt, "bk", kb)):
                for t2 in range(8):
                    wtile = wp.tile([128, 16, 256], F16, tag="w")
                    nc.sync.dma_start(wtile[:], wdram[t2, :, :, :])
                    for half in range(2):
                        t = 2 * t2 + half
                        psA = pp.tile([128, 512], F32, tag="pp")
                        psB = pp.tile([128, 512], F32, tag="pp")
                        for k in range(16):
                            w_ap = wtile[:, k, half * 128 : half * 128 + 128]
                            nc.tensor.matmul(
                                psA[:], w_ap, xa[k][:],
                                start=(k == 0), stop=(k == 15),
                            )
                            nc.tensor.matmul(
                                psB[:], w_ap, xb[k][:],
                                start=(k == 0), stop=(k == 15),
                            )
                        drain(dst[:, 0:512, t], psA[:], bias, t, "v")
                        drain(dst[:, 512:1024, t], psB[:], bias, t, "v")

            # --- V sweep B: positions 512-1023, carries attn pairs 0-31 ---
            feed(range(0, 32))
            for t2 in range(8):
                wtile = wp.tile([128, 16, 256], F16, tag="w")
                nc.sync.dma_start(wtile[:], wvt[t2, :, :, :])
                for half in range(2):
                    t = 2 * t2 + half
                    ps = pp.tile([128, 512], F32, tag="pp")
                    for k in range(16):
                        nc.tensor.matmul(
                            ps[:],
                            wtile[:, k, half * 128 : half * 128 + 128],
                            xb[k][:],
                            start=(k == 0), stop=(k == 15),
                        )
                        pump()
                    drain4(
                        [vbB[:, 128 * i : 128 * i + 128, t] for i in range(4)],
                        ps[:], "bv", t,
                    )

            # --- output projection, two sweeps over attO halves ---
            def fin_sweep(hh):
                drain_until(32 * hh + 32)
                if hh == 0:
                    feed(range(32, 64))
                for t2 in range(8):
                    wtile = wp.tile([128, 16, 256], F16, tag="w")
                    nc.sync.dma_start(wtile[:], wot[t2, :, :, :])
                    for half in range(2):
                        t = 2 * t2 + half
                        ps = pp.tile([128, 512], F32, tag="pp")
                        last = hh == 1 and t2 == 7 and half == 1
                        if last:
                            # column-split the final group so its drain and
                            # output DMA overlap the second half's matmuls
                            for colh in range(2):
                                c0 = 256 * colh
                                for sl in range(16):
                                    nc.tensor.matmul(
                                        ps[:, c0 : c0 + 256],
                                        wtile[:, sl, half * 128 : half * 128 + 128],
                                        attO[hh][:, sl, c0 : c0 + 256],
                                        start=(sl == 0), stop=(sl == 15),
                                    )
                                ob = op.tile([128, 256], F32, tag="obh")
                                drain(ob[:], ps[:, c0 : c0 + 256], "bo", t, "v")
                                nc.sync.dma_start(
                                    out[t, :, hh * 512 + c0 : hh * 512 + c0 + 256],
                                    ob[:],
                                )
                            continue
                        for sl in range(16):
                            nc.tensor.matmul(
                                ps[:],
                                wtile[:, sl, half * 128 : half * 128 + 128],
                                attO[hh][:, sl, :],
                                start=(sl == 0), stop=(sl == 15),
                            )
                            pump()
                        ob = op.tile([128, 512], F32, tag="ob")
                        if hh == 0:
                            drain4(
                                [ob[:, 128 * i : 128 * i + 128] for i in range(4)],
                                ps[:], "bo", t,
                            )
                        else:
                            drain(ob[:], ps[:], "bo", t, "v")
                        nc.sync.dma_start(
                            out[t, :, hh * 512 : hh * 512 + 512], ob[:]
                        )

            fin_sweep(0)
            drain_until(64)
            fin_sweep(1)

    nc.compile()
    return nc


def _get_nc():
    if "nc" not in _CACHE:
        _CACHE["nc"] = _build()
    return _CACHE["nc"]


def make_in_maps(inputs):
    x = np.ascontiguousarray(np.asarray(inputs["x"], dtype=np.float32))
    ws = {k: np.asarray(inputs[k], dtype=np.float32) for k in ("wq", "wk", "wv", "wo")}
    bs = {k: np.asarray(inputs[k], dtype=np.float32) for k in ("bq", "bk", "bv", "bo")}

    xf = x.reshape(B * S, E)
    f16 = lambda a: np.ascontiguousarray(a).astype(np.float16)

    def wtileize(w):
        # [t2, p, k, c] tiles: element (p, k, c) = w.T[k*128+p, t2*256+c]
        return np.ascontiguousarray(
            w.T.astype(np.float16).reshape(16, 128, 8, 256).transpose(2, 1, 0, 3)
        )

    btile = lambda b: np.ascontiguousarray(b.reshape(16, 128).T)
    ii = np.arange(128) // 16
    mask01 = (ii[:, None] == ii[None, :]).astype(np.float32)
    common = {
        "wqt": wtileize(ws["wq"]), "wkt": wtileize(ws["wk"]),
        "wvt": wtileize(ws["wv"]), "wot": wtileize(ws["wo"]),
        "bqt": btile(bs["bq"]), "bkt": btile(bs["bk"]),
        "bvt": btile(bs["bv"]), "bot": btile(bs["bo"]),
        "mask01": mask01, "ident": np.eye(128, dtype=np.float16),
    }
    in_maps = []
    for c in range(NCORES):
        xt_c = f16(xf[c * 1024 : (c + 1) * 1024].T).reshape(16, 128, 1024)
        in_maps.append({"xt": xt_c, **common})
    return in_maps


def assemble(results):
    out = np.empty((B, S, E), np.float32)
    for c in range(NCORES):
        O = results[c]["out"]  # [16 t, 128 p, 1024]; col = u*16 + h
        Oc = O.reshape(E, 64, 16)  # [j, u, h]
        tgt = out[c // 2].reshape(16, 128, E)
        v0 = (c % 2) * 64
        tgt[:, v0 : v0 + 64, :] = Oc.transpose(2, 1, 0)
    return out


def kernel(**inputs):
    global LAST_EXEC_NS
    nc = _get_nc()
    res = run_bass_kernel_spmd(nc, make_in_maps(inputs), core_ids=list(range(NCORES)))
    LAST_EXEC_NS = res.exec_time_ns
    return assemble(res.results)


# revision 44
# speedup vs baseline: 1.0854x; 1.0061x over previous
"""Trainium2 Bass kernel for nn_MultiHeadAttention_44281112822190.

8 NeuronCores, pure data parallelism over the 8192 (b,s) rows: core c takes
rows [c*1024, (c+1)*1024) (batch b = c//2, s-offset (c%2)*1024). No
collectives; the host shards inputs and reassembles the output.

Math notes:
  - The reference applies RoPE to q and k, then contracts q.k at the SAME
    position (per-position head-head attention [B,S,H,H]). RoPE is an
    orthogonal per-position rotation applied identically to q and k, so it
    cancels exactly in the scores: (R q).(R k) = q.k. The kernel skips RoPE
    entirely (freqs inputs are unused).
  - The reference's "h-major flatten" transpose(0,2,1,3).reshape(B,S,-1) is a
    scramble: out[b, h*128 + s//16, (s%16)*128 + d] = att_out[b, s, h, d].
    Each scrambled row draws from 16 consecutive positions of one head, all
    inside one core's shard, so the output projection stays core-local.

Numerics: all matmul operands are fp16 with fp32 PSUM accumulation ->
~7e-4 relative error end-to-end, 1 cycle/row on the PE.

Schedule (v2): the PE streaming floor for the four projections plus
attention is ~464 us/core; this version keeps TensorE dense:
  1. Startup: first wq tile + x^T in 16 fine chunks DMA'd first, weights
     pre-tiled on host so every weight DMA is contiguous (8 KB/partition
     runs). First matmul issues ~5 us in.
  2. Q, K projections: stationary = pre-tiled weight chunks, moving =
     x^T chunks; two N=512 matmuls per LDWEIGHTS; bias added during
     PSUM->SBUF drain. Layout [128 d, 1024 s, 16 h].
  3. V projection in two position-half sweeps (wv streamed twice).
     Attention pairs 0-31 (positions 0-511) are software-pipelined into
     sweep 2's matmul stream in three stages: A = scores matmul +
     exp/mask-sum/reciprocal/normalize, B = att+v transposes (PE), C =
     attO matmul + scatter. Stage lag (A->B 3 slots, B->C 2 slots) covers
     the Scalar/Vector/GpSimd chain latency so TensorE never waits.
  4. Output projection from merged attO halves [128 d, 16 sl, 512] with
     N=512 moving operands, in two sweeps; sweep 1 carries attention
     pairs 32-63 the same way. Drains alternate Vector/GpSimd in
     interleaved phases to keep Vector off the critical path.
Host reassembles the scrambled rows into the final [4, 2048, 2048] output.
"""

import os
import sys

sys.path.insert(0, "/opt/trn_rl_repo")

import numpy as np

import concourse.bacc as bacc
import concourse.mybir as mybir
import concourse.tile as tile
from concourse.bass_utils import run_bass_kernel_spmd

F32 = mybir.dt.float32
F16 = mybir.dt.float16
AF = mybir.ActivationFunctionType
ALU = mybir.AluOpType

B, S, E, H, D = 4, 2048, 2048, 16, 128
NCORES = 8
SCALE = 1.0 / float(np.sqrt(D))

_CACHE = {}
LAST_EXEC_NS = None


def _build():
    nc = bacc.Bacc(trn_type="TRN2", target_bir_lowering=False)

    xt = nc.dram_tensor("xt", [16, 128, 1024], F16, kind="ExternalInput")
    wqt = nc.dram_tensor("wqt", [8, 128, 16, 256], F16, kind="ExternalInput")
    wkt = nc.dram_tensor("wkt", [8, 128, 16, 256], F16, kind="ExternalInput")
    wvt = nc.dram_tensor("wvt", [8, 128, 16, 256], F16, kind="ExternalInput")
    wot = nc.dram_tensor("wot", [8, 128, 16, 256], F16, kind="ExternalInput")
    bqt = nc.dram_tensor("bqt", [128, 16], F32, kind="ExternalInput")
    bkt = nc.dram_tensor("bkt", [128, 16], F32, kind="ExternalInput")
    bvt = nc.dram_tensor("bvt", [128, 16], F32, kind="ExternalInput")
    bot = nc.dram_tensor("bot", [128, 16], F32, kind="ExternalInput")
    mask01 = nc.dram_tensor("mask01", [128, 128], F32, kind="ExternalInput")
    ident = nc.dram_tensor("ident", [128, 128], F16, kind="ExternalInput")
    out = nc.dram_tensor("out", [16, 128, 1024], F32, kind="ExternalOutput")

    with tile.TileContext(nc) as tc:
        with (
            tc.tile_pool(name="const", bufs=1) as cp,
            tc.tile_pool(name="xp", bufs=1) as xp,
            tc.tile_pool(name="qkv", bufs=1) as qkvp,
            tc.tile_pool(name="aop", bufs=1) as aop,
            tc.tile_pool(name="wp", bufs=3) as wp,
            tc.tile_pool(name="gp", bufs=5) as gp,
            tc.tile_pool(name="op", bufs=3) as op,
            tc.tile_pool(name="pp", bufs=4, space="PSUM") as pp,
            tc.tile_pool(name="pa", bufs=2, space="PSUM") as pa,
            tc.tile_pool(name="pb", bufs=1, space="PSUM") as pb,
        ):
            # first weight tile + x first-halves first: the V-A sweep needs
            # only wv[0] + 2 MB of x, so compute starts ~4 us in. wv[0] is
            # fetched in 4 pieces so its first k-chunks land ASAP.
            wv_first = wp.tile([128, 16, 256], F16, tag="w")
            for q in range(4):
                nc.sync.dma_start(
                    wv_first[:, 4 * q : 4 * q + 4, :], wvt[0, :, 4 * q : 4 * q + 4, :]
                )
            xa, xb = [], []
            wv_pre = [wv_first, None, None]
            for k in range(16):
                xc = xp.tile([128, 512], F16, tag=f"xa{k}", name=f"xa{k}")
                nc.sync.dma_start(xc[:], xt[k, :, 0:512])
                xa.append(xc)
                if k in (7, 15):
                    # keep all three weight-ring slots in flight from t=0:
                    # the slow DMA warmup otherwise starves V-A around t2=3
                    t2 = 1 if k == 7 else 2
                    wt = wp.tile([128, 16, 256], F16, tag="w")
                    nc.sync.dma_start(wt[:], wvt[t2, :, :, :])
                    wv_pre[t2] = wt

            bias_sb = {}
            for name, t_ in (("bq", bqt), ("bk", bkt), ("bv", bvt), ("bo", bot)):
                b_sb = cp.tile([128, 16], F32, tag=name)
                nc.sync.dma_start(b_sb[:], t_[:, :])
                bias_sb[name] = b_sb
            mask_sb = cp.tile([128, 128], F32, tag="mask")
            id_sb = cp.tile([128, 128], F16, tag="id")

            # xb tiles created here; their DMAs are issued inside the V-A
            # loop so they don't contend with the wv tile fetches V-A needs
            for k in range(16):
                xc = xp.tile([128, 512], F16, tag=f"xb{k}", name=f"xb{k}")
                xb.append(xc)

            qb = qkvp.tile([128, 1024, 16], F16, tag="qb")
            kb = qkvp.tile([128, 1024, 16], F16, tag="kb")
            vbA = qkvp.tile([128, 512, 16], F16, tag="vbA")
            vbB = qkvp.tile([128, 512, 16], F16, tag="vbB")
            attO = [
                aop.tile([128, 16, 512], F16, tag=f"attO{i}", name=f"attO{i}")
                for i in range(2)
            ]

            def drain(dst, ps, bias, t, eng):
                # GpSimd cannot read PSUM, so all drains go through Vector.
                nc.vector.tensor_scalar_add(dst, ps, bias_sb[bias][:, t : t + 1])

            def drain4(dst4, ps, bias, t):
                # chunked drain: keeps long Vector ops from delaying the
                # attention softmax chain during interleaved phases
                b_ap = bias_sb[bias][:, t : t + 1]
                for i in range(4):
                    nc.vector.tensor_scalar_add(
                        dst4[i], ps[:, 128 * i : 128 * i + 128], b_ap
                    )

            # --- attention stages -------------------------------------
            astate = {}

            def attn_A(p):
                G = 2 * p
                sc = pa.tile([128, 256], F32, tag="sc", bufs=2)
                for j in range(2):
                    s0 = (G + j) * 8
                    nc.tensor.matmul(
                        sc[:, 128 * j : 128 * j + 128],
                        qb[:, s0 : s0 + 8, :],
                        kb[:, s0 : s0 + 8, :],
                        start=True, stop=True,
                    )
                e2 = gp.tile([128, 256], F32, tag="e2")
                nc.scalar.activation(e2[:], sc[:, 0:256], AF.Exp, scale=SCALE)
                em2 = e2[:].rearrange("p (g c) -> p g c", g=2)
                den2 = gp.tile([128, 2], F32, tag="den2")
                for j, eng in ((0, nc.vector), (1, nc.vector)):
                    eng.scalar_tensor_tensor(
                        em2[:, j, :], e2[:, 128 * j : 128 * j + 128], 1.0,
                        mask_sb[:], ALU.bypass, ALU.mult,
                        accum_out=den2[:, j : j + 1],
                    )
                rec2 = gp.tile([128, 2], F32, tag="rec2")
                nc.vector.reciprocal(rec2[:], den2[:])
                att2 = gp.tile([128, 2, 128], F16, tag="att2")
                nc.gpsimd.tensor_tensor(
                    att2[:], em2,
                    rec2[:].unsqueeze(2).to_broadcast([128, 2, 128]),
                    ALU.mult,
                )
                astate[p] = {"att2": att2}

            def _b_transposes(p, j, tr):
                st = astate[p]
                vb_t, off = (vbA, 0) if p < 32 else (vbB, 512)
                s0 = (2 * p + j) * 8 - off
                nc.tensor.transpose(
                    tr[:, 128 * j : 128 * j + 128], st["att2"][:, j, :],
                    id_sb[:],
                )
                nc.tensor.transpose(
                    tr[:, 256 + 128 * j : 384 + 128 * j],
                    vb_t[:, s0 : s0 + 8, :], id_sb[:],
                )

            def attn_B1(p):
                tr = pb.tile([128, 512], F16, tag="tr")
                astate[p]["tr"] = tr
                _b_transposes(p, 0, tr)

            def attn_B2(p):
                tr = astate[p]["tr"]
                _b_transposes(p, 1, tr)
                trsb = gp.tile([128, 512], F16, tag="trsb")
                nc.scalar.activation(trsb[:], tr[:], AF.Copy)
                astate[p]["trsb"] = trsb

            def attn_C(p):
                st = astate.pop(p)
                trsb = st["trsb"]
                ao = pa.tile([128, 256], F32, tag="ao", bufs=1)
                for j in range(2):
                    nc.tensor.matmul(
                        ao[:, 128 * j : 128 * j + 128],
                        trsb[:, 256 + 128 * j : 384 + 128 * j],
                        trsb[:, 128 * j : 128 * j + 128],
                        start=True, stop=True,
                    )
                hh, u2 = p // 32, p % 32
                dst = attO[hh][:].rearrange(
                    "p (g2 i) (u h) -> p g2 i u h", g2=2, h=16
                )[:, :, :, u2, :]
                src = ao[:, 0:256].rearrange("p (g2 i h) -> p g2 i h", g2=2, i=8)
                nc.scalar.activation(dst, src, AF.Copy)

            # pipelined scheduler: one pump() per matmul slot (~213 ns).
            # Stage lags cover the cross-engine softmax chain latency so the
            # TensorE instructions never reach the head of the queue before
            # their operands are ready; spacing-2 keeps the small PSUM rings
            # (sc=2, ao=1, tr=1) from stalling back-to-back stages.
            pipe = {"slot": 0, "a": 0, "b1": 0, "b2": 0, "c": 0,
                    "a_slot": {}, "b1_slot": {}, "b2_slot": {},
                    "last": {"a": -9, "b1": -9, "b2": -9, "c": -9},
                    "pairs": []}

            def feed(pairs):
                pipe["pairs"].extend(pairs)

            def pump():
                s = pipe["slot"]
                P = pipe["pairs"]
                last = pipe["last"]
                if pipe["c"] < pipe["b2"]:
                    p = P[pipe["c"]]
                    if pipe["b2_slot"][p] <= s - 5 and last["c"] <= s - 2:
                        attn_C(p)
                        last["c"] = s
                        pipe["c"] += 1
                if pipe["b2"] < pipe["b1"]:
                    p = P[pipe["b2"]]
                    if pipe["b1_slot"][p] <= s - 2 and last["b2"] <= s - 2:
                        attn_B2(p)
                        pipe["b2_slot"][p] = s
                        last["b2"] = s
                        pipe["b2"] += 1
                if pipe["b1"] < pipe["a"] and pipe["b1"] - pipe["c"] < 5:
                    p = P[pipe["b1"]]
                    if pipe["a_slot"][p] <= s - 16 and last["b1"] <= s - 2:
                        attn_B1(p)
                        pipe["b1_slot"][p] = s
                        last["b1"] = s
                        pipe["b1"] += 1
                if (
                    pipe["a"] < len(P)
                    and pipe["a"] - pipe["b2"] < 5
                    and last["a"] <= s - 2
                ):
                    p = P[pipe["a"]]
                    attn_A(p)
                    pipe["a_slot"][p] = s
                    last["a"] = s
                    pipe["a"] += 1
                pipe["slot"] += 1

            def drain_until(n):
                # hard barrier: all attO writers up to pair n issued before
                # any consumer of their columns is enqueued
                while pipe["c"] < n:
                    pump()

            # --- V sweep A: positions 0-511, dense, runs first (needs only
            # wv[0] + the x first-halves, so compute starts early) ---
            for t2 in range(8):
                if t2 < 3:
                    wtile = wv_pre[t2]
                else:
                    wtile = wp.tile([128, 16, 256], F16, tag="w")
                    nc.sync.dma_start(wtile[:], wvt[t2, :, :, :])
                if t2 == 2:
                    # mask/ident aren't needed until attention (~280 us in)
                    nc.sync.dma_start(mask_sb[:], mask01[:, :])
                    nc.sync.dma_start(id_sb[:], ident[:, :])
                if 3 <= t2 <= 6:
                    for k in range(4 * (t2 - 3), 4 * (t2 - 3) + 4):
                        nc.sync.dma_start(xb[k][:], xt[k, :, 512:1024])
                for half in range(2):
                    t = 2 * t2 + half
                    ps = pp.tile([128, 512], F32, tag="pp")
                    for k in range(16):
                        nc.tensor.matmul(
                            ps[:], wtile[:, k, half * 128 : half * 128 + 128],
                            xa[k][:],
                            start=(k == 0), stop=(k == 15),
                        )
                    drain(vbA[:, :, t], ps[:], "bv", t, "v")

            # --- Q, K projections -> [128 d, 1024 s, 16 h] fp16 ---
            for wdram, bias, dst in ((wqt, "bq", qb), (wkt, "bk", kb)):
                for t2 in range(8):
                    wtile = wp.tile([128, 16, 256], F16, tag="w")
                    nc.sync.dma_start(wtile[:], wdram[t2, :, :, :])
                    for half in range(2):
                        t = 2 * t2 + half
                        psA = pp.tile([128, 512], F32, tag="pp")
                        psB = pp.tile([128, 512], F32, tag="pp")
                        for k in range(16):
                            w_ap = wtile[:, k, half * 128 : half * 128 + 128]
                            nc.tensor.matmul(
                                psA[:], w_ap, xa[k][:],
                                start=(k == 0), stop=(k == 15),
                            )
                            nc.tensor.matmul(
                                psB[:], w_ap, xb[k][:],
                                start=(k == 0), stop=(k == 15),
                            )
                        drain(dst[:, 0:512, t], psA[:], bias, t, "v")
                        drain(dst[:, 512:1024, t], psB[:], bias, t, "v")

            # --- V sweep B: positions 512-1023, carries attn pairs 0-31 ---
            feed(range(0, 32))
            for t2 in range(8):
                wtile = wp.tile([128, 16, 256], F16, tag="w")
                nc.sync.dma_start(wtile[:], wvt[t2, :, :, :])
                for half in range(2):
                    t = 2 * t2 + half
                    ps = pp.tile([128, 512], F32, tag="pp")
                    for k in range(16):
                        nc.tensor.matmul(
                            ps[:],
                            wtile[:, k, half * 128 : half * 128 + 128],
                            xb[k][:],
                            start=(k == 0), stop=(k == 15),
                        )
                        pump()
                    drain4(
                        [vbB[:, 128 * i : 128 * i + 128, t] for i in range(4)],
                        ps[:], "bv", t,
                    )

            # --- output projection, two sweeps over attO halves ---
            def fin_sweep(hh):
                drain_until(32 * hh + 32)
                if hh == 0:
                    feed(range(32, 64))
                for t2 in range(8):
                    wtile = wp.tile([128, 16, 256], F16, tag="w")
                    nc.sync.dma_start(wtile[:], wot[t2, :, :, :])
                    for half in range(2):
                        t = 2 * t2 + half
                        ps = pp.tile([128, 512], F32, tag="pp")
                        last = hh == 1 and t2 == 7 and half == 1
                        if last:
                            # column-split the final group so its drain and
                            # output DMA overlap the second half's matmuls
                            for colh in range(2):
                                c0 = 256 * colh
                                for sl in range(16):
                                    nc.tensor.matmul(
                                        ps[:, c0 : c0 + 256],
                                        wtile[:, sl, half * 128 : half * 128 + 128],
                                        attO[hh][:, sl, c0 : c0 + 256],
                                        start=(sl == 0), stop=(sl == 15),
                                    )
                                ob = op.tile([128, 256], F32, tag="obh")
                                drain(ob[:], ps[:, c0 : c0 + 256], "bo", t, "v")
                                nc.sync.dma_start(
                                    out[t, :, hh * 512 + c0 : hh * 512 + c0 + 256],
                                    ob[:],
                                )
                            continue
                        for sl in range(16):
                            nc.tensor.matmul(
                                ps[:],
                                wtile[:, sl, half * 128 : half * 128 + 128],
                                attO[hh][:, sl, :],
                                start=(sl == 0), stop=(sl == 15),
                            )
                            pump()
                        ob = op.tile([128, 512], F32, tag="ob")
                        if hh == 0:
                            drain4(
                                [ob[:, 128 * i : 128 * i + 128] for i in range(4)],
                                ps[:], "bo", t,
                            )
                        else:
                            drain(ob[:], ps[:], "bo", t, "v")
                        nc.sync.dma_start(
                            out[t, :, hh * 512 : hh * 512 + 512], ob[:]
                        )

            fin_sweep(0)
            drain_until(64)
            fin_sweep(1)

    nc.compile()
    return nc


def _get_nc():
    if "nc" not in _CACHE:
        _CACHE["nc"] = _build()
    return _CACHE["nc"]


def make_in_maps(inputs):
    x = np.ascontiguousarray(np.asarray(inputs["x"], dtype=np.float32))
    ws = {k: np.asarray(inputs[k], dtype=np.float32) for k in ("wq", "wk", "wv", "wo")}
    bs = {k: np.asarray(inputs[k], dtype=np.float32) for k in ("bq", "bk", "bv", "bo")}

    xf = x.reshape(B * S, E)
    f16 = lambda a: np.ascontiguousarray(a).astype(np.float16)

    def wtileize(w):
        # [t2, p, k, c] tiles: element (p, k, c) = w.T[k*128+p, t2*256+c]
        return np.ascontiguousarray(
            w.T.astype(np.float16).reshape(16, 128, 8, 256).transpose(2, 1, 0, 3)
        )

    btile = lambda b: np.ascontiguousarray(b.reshape(16, 128).T)
    ii = np.arange(128) // 16
    mask01 = (ii[:, None] == ii[None, :]).astype(np.float32)
    common = {
        "wqt": wtileize(ws["wq"]), "wkt": wtileize(ws["wk"]),
        "wvt": wtileize(ws["wv"]), "wot": wtileize(ws["wo"]),
        "bqt": btile(bs["bq"]), "bkt": btile(bs["bk"]),
        "bvt": btile(bs["bv"]), "bot": btile(bs["bo"]),
        "mask01": mask01, "ident": np.eye(128, dtype=np.float16),
    }
    in_maps = []
    for c in range(NCORES):
        xt_c = f16(xf[c * 1024 : (c + 1) * 1024].T).reshape(16, 128, 1024)
        in_maps.append({"xt": xt_c, **common})
    return in_maps


def assemble(results):
    out = np.empty((B, S, E), np.float32)
    for c in range(NCORES):
        O = results[c]["out"]  # [16 t, 128 p, 1024]; col = u*16 + h
        Oc = O.reshape(E, 64, 16)  # [j, u, h]
        tgt = out[c // 2].reshape(16, 128, E)
        v0 = (c % 2) * 64
        tgt[:, v0 : v0 + 64, :] = Oc.transpose(2, 1, 0)
    return out


def kernel(**inputs):
    global LAST_EXEC_NS
    nc = _get_nc()
    res = run_bass_kernel_spmd(nc, make_in_maps(inputs), core_ids=list(range(NCORES)))
    LAST_EXEC_NS = res.exec_time_ns
    return assemble(res.results)
